# revision 1
# baseline (speedup 1.0000x reference)
"""Size-weighted focal loss on 8 Trainium2 NeuronCores.

Math (per element, x = logit, t in {0,1}):
  w  = x*(1-2t)            (so sigmoid(w) = p_t-complement path)
  L  = softplus(w)         = -log(pt)  (stable BCE)
  s2 = sigmoid(w)^2        = (1-pt)^2
  a  = 0.75 - 0.5*t        (alpha_t with ALPHA=0.25)
  elem = a * s2 * L

Device formulation (per core, 8 samples of [128,2048]):
  tf = float(t)
  v  = (tf - 0.5) * x      (= w'/2 where w' = -w)
  E  = exp(2v);  N = ln(E+1) (= softplus(-w));  s2 = exp(-2N)
  a  = tf*(-0.5)+0.75 ;  F = a*s2
  per-sample loss sum = Sum F*(N - 2v)   [since L = N - w' = N - 2v]
    computed on PE: psum[128,256] += F_chunk^T @ [N_chunk | v_chunk],
    then diag extraction with mask M[m,m]=+1, M[m,128+m]=-2.
  fg = Sum tf recovered from Sum a: fg = 2*(0.75*262144 - Sum a).

Host combines per-sample sums: mean_b( (S_b/HW) * sw(fg_b) ).
"""

import functools
import numpy as np
from contextlib import ExitStack

P = 128
B_PER_CORE = 8
N_CORES = 8
H = 512
W = 512
HW = H * W                 # 262144
FREE = HW // P             # 2048
NCHUNK = FREE // P         # 16

_GLOBAL = {}


def _build():
    import concourse.bacc as bacc
    import concourse.tile as tile
    import concourse.mybir as mybir

    f32 = mybir.dt.float32
    i32 = mybir.dt.int32
    Alu = mybir.AluOpType
    Act = mybir.ActivationFunctionType

    nc = bacc.Bacc("TRN2", target_bir_lowering=False, debug=False,
                   num_devices=N_CORES)

    pred_in = nc.dram_tensor("pred", (B_PER_CORE, H, W), f32, kind="ExternalInput")
    targ_in = nc.dram_tensor("target", (B_PER_CORE, H, W), i32, kind="ExternalInput")
    mask_in = nc.dram_tensor("mask", (P, 2 * P), f32, kind="ExternalInput")
    out_t = nc.dram_tensor("out", (B_PER_CORE, 2), f32, kind="ExternalOutput")

    # [b, 512, 512] -> [b, 128, 2048]; partition p holds contiguous 2048 elems
    x_v = pred_in.ap().rearrange("b (p q) w -> b p (q w)", p=P)
    t_v = targ_in.ap().rearrange("b (p q) w -> b p (q w)", p=P)

    with ExitStack() as ctx:
        tc = ctx.enter_context(tile.TileContext(nc))
        singles = ctx.enter_context(tc.tile_pool(name="singles", bufs=1))
        io = ctx.enter_context(tc.tile_pool(name="io", bufs=3))
        work = ctx.enter_context(tc.tile_pool(name="work", bufs=2))
        psum = ctx.enter_context(tc.tile_pool(name="psum", bufs=4, space="PSUM"))
        psum_fin = ctx.enter_context(tc.tile_pool(name="psum_fin", bufs=1, space="PSUM"))

        mask_t = singles.tile([P, 2 * P], f32)
        nc.sync.dma_start(out=mask_t[:], in_=mask_in.ap())
        ones_t = singles.tile([P, 1], f32)
        nc.vector.memset(ones_t[:], 1.0)
        Scol = singles.tile([P, B_PER_CORE], f32)   # per-partition loss partials
        Acol = singles.tile([P, B_PER_CORE], f32)   # per-partition sum(a) partials

        for b in range(B_PER_CORE):
            xt = io.tile([P, FREE], f32, tag="xt")
            tt = io.tile([P, FREE], i32, tag="tt")
            nc.sync.dma_start(out=xt[:], in_=x_v[b])
            nc.sync.dma_start(out=tt[:], in_=t_v[b])

            tf = work.tile([P, FREE], f32, tag="tf")
            nc.vector.tensor_copy(tf[:], tt[:])              # i32 -> f32

            # nw holds both PE rhs blocks: [:,0,:] = N, [:,1,:] = v
            nw = work.tile([P, 2, FREE], f32, tag="nw")
            Nt = nw[:, 0, :]
            vt = nw[:, 1, :]
            # v = (tf - 0.5) * x
            nc.vector.scalar_tensor_tensor(
                out=vt, in0=tf[:], scalar=0.5, in1=xt[:],
                op0=Alu.subtract, op1=Alu.mult)

            Et = work.tile([P, FREE], f32, tag="Et")
            nc.scalar.activation(Et[:], vt, Act.Exp, scale=2.0)

            nc.scalar.activation(Nt, Et[:], Act.Ln, bias=1.0)

            s2 = work.tile([P, FREE], f32, tag="s2")
            nc.scalar.activation(s2[:], Nt[:], Act.Exp, scale=-2.0)

            # accum_out changes semantics: at = tf*(-0.5) elementwise, and
            # Acol[p] = sum_f(at) + 0.75 (op1/scalar2 act on the reduction)
            at = work.tile([P, FREE], f32, tag="at")
            nc.vector.tensor_scalar(at[:], tf[:], -0.5, 0.75,
                                    Alu.mult, Alu.add,
                                    accum_out=Acol[:, b:b + 1])

            # F = (at + 0.75) * s2 = (0.75 - 0.5*tf) * s2 = a * s2
            Ft = work.tile([P, FREE], f32, tag="Ft")
            nc.vector.scalar_tensor_tensor(
                out=Ft[:], in0=at[:], scalar=0.75, in1=s2[:],
                op0=Alu.add, op1=Alu.mult)

            ps = psum.tile([P, 2 * P], f32)
            for c in range(NCHUNK):
                sl = slice(c * P, (c + 1) * P)
                # rhs = [N_chunk | v_chunk] as one [128, 2, 128] AP ->
                # psum cols 0:128 = F^T N, cols 128:256 = F^T v
                nc.tensor.matmul(ps[:], Ft[:, sl], nw[:, :, sl],
                                 start=(c == 0), stop=(c == NCHUNK - 1))

            scr = work.tile([P, 2 * P], f32, tag="scr")
            # Scol[:,b] = sum_c ps[:,c] * mask[:,c]  (diag picks +N, -2v blocks)
            nc.vector.scalar_tensor_tensor(
                out=scr[:], in0=ps[:], scalar=0.0, in1=mask_t[:],
                op0=Alu.add, op1=Alu.mult,
                accum_out=Scol[:, b:b + 1])

        # Final partition reduction via ones-matmul: [128,8]^T @ [128,1] -> [8,1]
        fin = psum_fin.tile([B_PER_CORE, 2], f32)
        nc.tensor.matmul(fin[:, 0:1], Scol[:], ones_t[:], start=True, stop=True)
        nc.tensor.matmul(fin[:, 1:2], Acol[:], ones_t[:], start=True, stop=True)
        out_sb = singles.tile([B_PER_CORE, 2], f32)
        nc.vector.tensor_copy(out_sb[:], fin[:])
        nc.sync.dma_start(out=out_t.ap(), in_=out_sb[:])

    nc.compile()
    return nc


def _get_nc():
    if "nc" not in _GLOBAL:
        _GLOBAL["nc"] = _build()
    return _GLOBAL["nc"]


def _mask_np():
    m = np.zeros((P, 2 * P), dtype=np.float32)
    idx = np.arange(P)
    m[idx, idx] = 1.0
    m[idx, P + idx] = -2.0
    return m


GAMMA = 2.0
ALPHA = 0.25
SIZE_POWER = 0.5


def kernel(pred: np.ndarray, target: np.ndarray) -> np.ndarray:
    from concourse import bass_utils

    nc = _get_nc()
    pred = np.ascontiguousarray(np.asarray(pred, dtype=np.float32))
    target = np.ascontiguousarray(np.asarray(target, dtype=np.int32))
    B = pred.shape[0]
    mask = _mask_np()

    in_maps = []
    for i in range(N_CORES):
        sl = slice(i * B_PER_CORE, (i + 1) * B_PER_CORE)
        in_maps.append({
            "pred": np.ascontiguousarray(pred[sl, 0]),
            "target": np.ascontiguousarray(target[sl]),
            "mask": mask,
        })

    res = bass_utils.run_bass_kernel_spmd(
        nc, in_maps, core_ids=list(range(N_CORES)),
        trace=bool(_GLOBAL.get("trace", False)),
        **_GLOBAL.get("run_kwargs", {}),
    )
    _GLOBAL["last_results"] = res

    outs = np.concatenate([r["out"] for r in res.results], axis=0)  # [64, 2]
    S = outs[:, 0].astype(np.float64)          # per-sample sum(a*s2*L)
    A = outs[:, 1].astype(np.float64)          # per-sample sum(-0.5*tf) + 96
    fg = 2.0 * (96.0 - A)                      # per-sample foreground count
    fg = np.rint(fg)
    sw = np.where(fg > 0,
                  np.minimum(100.0 / np.power(np.maximum(fg, 1.0), SIZE_POWER), 10.0),
                  1.0)
    per_sample = (S / HW) * sw
    return np.float32(per_sample.mean())

